# revision 1
# baseline (speedup 1.0000x reference)
"""Trainium2 Bass kernel for nn_Net4 (hypernetwork RNN scan).

Model (per step t, per batch row b):
  h1 = sigmoid(m @ A1 + pre1[t])          A1 = W_enc_w[:64]
  h2 = sigmoid(m @ B1 + pre2[t])          B1 = b_enc_w[:64]
  Wm = (h1 @ W_dec_w + W_dec_b).reshape(64,64)
  bm = h2 @ b_dec_w + b_dec_b
  m' = sigmoid(Wm @ m + bm)
  loss[t] = (logsumexp(m'@dec_w+dec_b) - (m'@dec_w+dec_b)[y]) / ln2

pre1/pre2 are the window-dependent parts, precomputed on device via a
shifted-embedding matmul.  The bilinear Wm@m is reassociated as
  a[b,i] = sum_h h1[b,h] * T[b,h,i],  T[b,h,i] = sum_j W2r[h,i,j] m[b,j]
T is produced by 32 weight-stationary matmuls (chunk c covers i=c and
i=c+32 in the two partition halves), then contracted with h1 by 4 tiny
matmuls reading strided slices of T from SBUF.

Sharding: batch rows 2k,2k+1 -> core k; zero cross-core communication.
"""

import os
import sys
import numpy as np

sys.path.insert(0, "/opt/trn_rl_repo")

import concourse.bass as bass
import concourse.bacc as bacc
import concourse.mybir as mybir
import concourse.tile as tile
from concourse.bass_utils import run_bass_kernel_spmd

import ml_dtypes

BF16 = ml_dtypes.bfloat16

Cin, E, L, M, H, Cout = 256, 16, 64, 64, 64, 256
B, N = 16, 2048
D = M + L * E  # 1088
NCORES = 8
BL = B // NCORES  # 2 batch rows per core
NB = N * BL       # 4096 (t,b) pairs per core
TAU = N + L - 8   # e8 time length: tau in [0, 2104)
E8COLS = TAU * BL  # 4208

F32 = mybir.dt.float32
BF16_DT = mybir.dt.bfloat16
AF = mybir.ActivationFunctionType

_cache = {}


def _build_nc(unroll=16, staggered=False):
    nc = bacc.Bacc("TRN2", target_bir_lowering=False, debug=True)

    # ---- DRAM parameters (per-core inputs) ----
    def P(name, shape, dt):
        return nc.declare_dram_parameter(name, list(shape), dt, isOutput=False)

    e8_d = P("e8", (128, E8COLS), BF16_DT)
    wpre1_d = P("wpre1", (128, 8 * 128), BF16_DT)
    wpre2_d = P("wpre2", (128, 8 * 64), BF16_DT)
    bias1_d = P("bias1", (1, 128), BF16_DT)   # [W_enc_b | W_enc_b]
    bias2_d = P("bias2", (1, 64), BF16_DT)    # b_enc_b
    a1b1_d = P("a1b1", (128, 192), BF16_DT)
    wstatT_d = P("wstatT", (128, 16 * 128), BF16_DT)
    wbT_d = P("wbT", (64, 64), F32)           # W_dec_b reshaped [j,i]
    bw65_d = P("bw65", (65, 64), F32)         # [b_dec_w ; b_dec_b]
    decstat_d = P("decstat", (65, 256), F32)  # [dec_w ; dec_b]
    gaug_d = P("gaug", (65, NB), F32)
    ones65_d = P("ones65", (65, 1), F32)
    ones128_d = P("ones128", (128, 1), F32)
    out_d = nc.declare_dram_parameter("out", [1, NB], F32, isOutput=True)

    with tile.TileContext(nc) as tc:
        with (
            tc.tile_pool(name="persist", bufs=1) as pp,
            tc.tile_pool(name="psum", bufs=2, space="PSUM") as psp,
        ):
            e8 = pp.tile([128, E8COLS], BF16_DT)
            wpre1 = pp.tile([128, 8 * 128], BF16_DT)
            wpre2 = pp.tile([128, 8 * 64], BF16_DT)
            bias1 = pp.tile([1, 128], BF16_DT)
            bias2 = pp.tile([1, 64], BF16_DT)
            a1b1 = pp.tile([128, 192], BF16_DT)
            wstatT = pp.tile([128, 16 * 128], BF16_DT)
            wbT = pp.tile([64, 64], F32)
            bw65 = pp.tile([65, 64], F32)
            decstat = pp.tile([65, 256], F32)
            gaug = pp.tile([65, NB], F32)
            ones65 = pp.tile([65, 1], F32)
            ones128 = pp.tile([128, 1], F32)

            for sb, dr in [
                (e8, e8_d), (wpre1, wpre1_d), (wpre2, wpre2_d),
                (bias1, bias1_d), (bias2, bias2_d), (a1b1, a1b1_d),
                (wstatT, wstatT_d), (wbT, wbT_d),
                (bw65, bw65_d), (decstat, decstat_d),
                (gaug, gaug_d), (ones65, ones65_d), (ones128, ones128_d),
            ]:
                nc.default_dma_engine.dma_start(sb[:], dr[:])

            pre1 = pp.tile([128, NB], F32)
            pre2 = pp.tile([64, NB], F32)
            m_hist = pp.tile([65, NB + 2 * BL], F32)  # row 64 == 1.0
            m_bf = pp.tile([128, BL], BF16_DT)
            h2t = pp.tile([65, BL], F32)              # row 64 == 1.0
            onerow = pp.tile([1, 512], BF16_DT)

            nc.vector.memset(m_hist[0:64, 0:BL], 0.0)
            nc.vector.memset(m_hist[64:65, :], 1.0)
            nc.vector.memset(m_bf[:], 0.0)  # both halves
            nc.vector.memset(h2t[64:65, :], 1.0)
            nc.vector.memset(onerow[:], 1.0)

            # ---- precompute pre1/pre2 ----
            for n in range(8):
                ps1 = psp.tile([128, 512], F32, tag="pps")
                for c in range(8):
                    nc.tensor.matmul(
                        ps1[:],
                        wpre1[:, c * 128:(c + 1) * 128],
                        e8[:, 16 * c + 512 * n: 16 * c + 512 * n + 512],
                        start=(c == 0), stop=False,
                    )
                nc.tensor.matmul(ps1[:], bias1[:], onerow[:],
                                 start=False, stop=True, skip_group_check=True)
                nc.vector.tensor_copy(pre1[:, 512 * n: 512 * (n + 1)], ps1[:])
                ps2 = psp.tile([64, 512], F32, tag="pps")
                for c in range(8):
                    nc.tensor.matmul(
                        ps2[:],
                        wpre2[:, c * 64:(c + 1) * 64],
                        e8[:, 16 * c + 512 * n: 16 * c + 512 * n + 512],
                        start=(c == 0), stop=False,
                    )
                nc.tensor.matmul(ps2[:], bias2[:], onerow[:],
                                 start=False, stop=True, skip_group_check=True)
                nc.vector.tensor_copy(pre2[:, 512 * n: 512 * (n + 1)], ps2[:])

            # ---- the scan ----
            with (
                tc.tile_pool(name="scan_sb", bufs=2) as wp,
                tc.For_i(0, N, unroll, staggered_reset=staggered,
                         hint_engines=(mybir.EngineType.PE,)) as iv,
            ):
                for k in range(unroll):
                    tcol = (iv + k) * BL
                    g_ps = psp.tile([128, 4], F32, tag="g_ps")
                    T_pse = psp.tile([128, 16, BL], F32, tag="T_pse", bufs=1)
                    T_pso = psp.tile([128, 16, BL], F32, tag="T_pso", bufs=1)
                    a_ps = psp.tile([64, BL], F32, tag="a_ps")
                    h1p = wp.tile([128, BL], F32, tag="h1p")
                    h2p = wp.tile([64, BL], F32, tag="h2p")
                    h1d = wp.tile([128, BL], F32, tag="h1d")
                    tsb = wp.tile([128, 32, BL], F32, tag="tsb")

                    # g row-paired: h1-preact (dup) on rows 0-63, h2 on 64-127
                    nc.tensor.matmul(g_ps[:, 0:2], a1b1[0:64, 0:128],
                                     m_bf[0:64, :], start=True, stop=True,
                                     tile_position=(0, 0))
                    nc.tensor.matmul(g_ps[0:64, 2:4], a1b1[64:128, 128:192],
                                     m_bf[64:128, :], start=True, stop=True,
                                     tile_position=(64, 0))
                    # T chunks, interleaved across row groups
                    for p2 in range(16):
                        nc.tensor.matmul(
                            T_pse[:, p2, :],
                            wstatT[0:64, p2 * 128:(p2 + 1) * 128],
                            m_bf[0:64, :], start=True, stop=True,
                            tile_position=(0, 0))
                        nc.tensor.matmul(
                            T_pso[:, p2, :],
                            wstatT[64:128, p2 * 128:(p2 + 1) * 128],
                            m_bf[64:128, :], start=True, stop=True,
                            tile_position=(64, 0))
                    # h = sigmoid(g + pre)
                    nc.vector.tensor_tensor(h1p[:], g_ps[:, 0:2],
                                            pre1[:, bass.ds(tcol, BL)],
                                            mybir.AluOpType.add)
                    nc.scalar.activation(h1d[:], h1p[:], AF.Sigmoid)
                    nc.vector.tensor_tensor(h2p[:], g_ps[0:64, 2:4],
                                            pre2[:, bass.ds(tcol, BL)],
                                            mybir.AluOpType.add)
                    nc.scalar.activation(h2t[0:64, :], h2p[:], AF.Sigmoid)
                    # T -> SBUF: tsb[:, c, :] with c = 2*p2 + odd
                    nc.vector.tensor_copy(tsb[:, 0:32:2, :], T_pse[:])
                    nc.vector.tensor_copy(tsb[:, 1:32:2, :], T_pso[:])
                    # a = WbT@m + bw65@h2 + sum_h h1*T
                    nc.tensor.matmul(a_ps[:], wbT[:],
                                     m_hist[0:64, bass.ds(tcol, BL)],
                                     start=True, stop=False)
                    nc.tensor.matmul(a_ps[:], bw65[:], h2t[:],
                                     start=False, stop=False, skip_group_check=True)
                    for b in range(BL):
                        nc.tensor.matmul(a_ps[0:32, b: b + 1],
                                         tsb[0:64, :, b], h1d[0:64, b: b + 1],
                                         start=False, stop=False,
                                         skip_group_check=True,
                                         tile_position=(0, 0))
                        last = b == BL - 1
                        nc.tensor.matmul(a_ps[32:64, b: b + 1],
                                         tsb[64:128, :, b], h1d[64:128, b: b + 1],
                                         start=False, stop=last,
                                         skip_group_check=True,
                                         tile_position=(64, 32))
                    # m' = sigmoid(a): bf16 dup halves first (critical), then f32 hist
                    nc.scalar.activation(m_bf[0:64, :], a_ps[:], AF.Sigmoid)
                    nc.scalar.activation(m_bf[64:128, :], a_ps[:], AF.Sigmoid)
                    nc.scalar.activation(m_hist[0:64, bass.ds(tcol + BL, BL)],
                                         a_ps[:], AF.Sigmoid)

            # ---- bulk loss ----
            lse = pp.tile([1, NB], F32)
            paug = pp.tile([65, NB], F32)
            loss = pp.tile([1, NB], F32)
            nc.vector.tensor_copy(paug[64:65, :], gaug[64:65, :])
            nc.vector.tensor_tensor(paug[0:64, :], gaug[0:64, :],
                                    m_hist[0:64, BL: NB + BL],
                                    mybir.AluOpType.mult)
            with tc.tile_pool(name="bulk", bufs=2) as bp:
                for tcn in range(8):
                    sl = slice(512 * tcn, 512 * (tcn + 1))
                    se_ps = psp.tile([1, 512], F32, tag="pps")
                    for half in range(2):
                        lg_ps = psp.tile([128, 512], F32, tag="pps")
                        exps = bp.tile([128, 512], F32, tag="exps")
                        nc.tensor.matmul(
                            lg_ps[:],
                            decstat[:, half * 128:(half + 1) * 128],
                            m_hist[:, BL + 512 * tcn: BL + 512 * (tcn + 1)],
                            start=True, stop=True)
                        nc.scalar.activation(exps[:], lg_ps[:], AF.Exp)
                        nc.tensor.matmul(se_ps[:], ones128[:], exps[:],
                                         start=(half == 0), stop=(half == 1))
                    nc.scalar.activation(lse[:, sl], se_ps[:], AF.Ln)
                    pk_ps = psp.tile([1, 512], F32, tag="pps")
                    nc.tensor.matmul(pk_ps[:], ones65[:], paug[:, sl],
                                     start=True, stop=True)
                    nc.vector.tensor_tensor(loss[:, sl], lse[:, sl], pk_ps[:],
                                            mybir.AluOpType.subtract)
            nc.vector.tensor_scalar_mul(loss[:], loss[:],
                                        float(1.0 / np.log(2.0)))
            nc.default_dma_engine.dma_start(out_d[:], loss[:])

    nc.compile()
    return nc


def _prep_core_inputs(x0, emb, W_enc_w, W_enc_b, W_dec_w, W_dec_b,
                      b_enc_w, b_enc_b, b_dec_w, b_dec_b, dec_w, dec_b):
    """Host-side gathers/packing -> list of per-core input dicts."""
    f32 = np.float32
    x0 = np.asarray(x0)
    xp = np.concatenate([np.zeros((B, L), x0.dtype), x0], axis=1)  # [B, N+L]
    e = np.asarray(emb, f32)[xp]  # [B, N+L, E]

    # shared weight packs
    Wcat = np.concatenate([np.asarray(W_enc_w, f32), np.asarray(b_enc_w, f32)],
                          axis=1)  # [1088, 128]
    wpre1 = np.zeros((128, 8 * 128), f32)
    wpre2 = np.zeros((128, 8 * 64), f32)
    for c in range(8):
        blk = Wcat[64 + 128 * c: 64 + 128 * (c + 1)]  # [128, 128]
        wpre1[:, c * 128: c * 128 + 64] = blk[:, :64]
        wpre1[:, c * 128 + 64: c * 128 + 128] = blk[:, :64]
        wpre2[:, c * 64:(c + 1) * 64] = blk[:, 64:]
    bias1 = np.concatenate([np.asarray(W_enc_b, f32)] * 2).reshape(1, 128)
    bias2 = np.asarray(b_enc_b, f32).reshape(1, 64)
    a1b1 = np.zeros((128, 192), f32)
    a1b1[0:64, 0:128] = np.concatenate([Wcat[:64, :64]] * 2, axis=1)
    a1b1[64:128, 128:192] = Wcat[:64, 64:]

    W2r = np.asarray(W_dec_w, f32).reshape(H, M, M)  # [h, i, j]
    wstatT = np.zeros((128, 16 * 128), f32)
    for p2 in range(16):
        for half, c in ((0, 2 * p2), (1, 2 * p2 + 1)):
            rows = slice(64 * half, 64 * half + 64)
            wstatT[rows, p2 * 128: p2 * 128 + 64] = W2r[:, c, :].T
            wstatT[rows, p2 * 128 + 64: p2 * 128 + 128] = W2r[:, c + 32, :].T
    wbT = np.asarray(W_dec_b, f32).reshape(M, M).T.copy()  # [j, i]
    bw65 = np.concatenate([np.asarray(b_dec_w, f32),
                           np.asarray(b_dec_b, f32).reshape(1, 64)], axis=0)
    decstat = np.concatenate([np.asarray(dec_w, f32),
                              np.asarray(dec_b, f32).reshape(1, 256)], axis=0)
    ones65 = np.ones((65, 1), f32)
    ones128 = np.ones((128, 1), f32)

    shared = dict(
        wpre1=wpre1.astype(BF16), wpre2=wpre2.astype(BF16),
        bias1=bias1.astype(BF16), bias2=bias2.astype(BF16),
        a1b1=a1b1.astype(BF16),
        wstatT=wstatT.astype(BF16), wbT=wbT, bw65=bw65,
        decstat=decstat, ones65=ones65, ones128=ones128,
    )

    in_maps = []
    dec_wT = np.asarray(dec_w, f32).T.copy()  # [256, 64]
    dec_bv = np.asarray(dec_b, f32)
    for k in range(NCORES):
        rows = slice(BL * k, BL * (k + 1))
        ek = e[rows]  # [BL, N+L, E]
        # e8[l_sub*16+eps, tau*BL+b] = ek[b, tau+l_sub, eps]
        e8 = np.zeros((128, E8COLS), f32)
        for ls in range(8):
            # [BL, TAU, E] -> [E? ] want [eps, tau, b]
            blk = ek[:, ls: ls + TAU, :].transpose(2, 1, 0)  # [E, TAU, BL]
            e8[ls * 16:(ls + 1) * 16] = blk.reshape(E, E8COLS)
        y = np.asarray(x0[rows])  # [BL, N]
        g = dec_wT[y]  # [BL, N, 64]
        gaug = np.zeros((65, NB), f32)
        gaug[:64] = g.transpose(2, 1, 0).reshape(64, NB)
        gaug[64] = dec_bv[y].T.reshape(NB)
        d = dict(shared)
        d["e8"] = e8.astype(BF16)
        d["gaug"] = gaug
        in_maps.append(d)
    return in_maps


def kernel(**inputs):
    key = "nc"
    if key not in _cache:
        _cache[key] = _build_nc()
    nc = _cache[key]
    in_maps = _prep_core_inputs(**inputs)
    res = run_bass_kernel_spmd(nc, in_maps, list(range(NCORES)),
                               trace=bool(os.environ.get("KERNEL_TRACE")))
    _cache["last_result"] = res
    out = np.zeros((N, B), np.float32)
    for k in range(NCORES):
        out[:, BL * k: BL * (k + 1)] = res.results[k]["out"].reshape(N, BL)
    return out.reshape(-1)



# revision 6
# speedup vs baseline: 1.0006x; 1.0006x over previous
"""Trainium2 Bass kernel for nn_Net4 (hypernetwork RNN scan).

Model (per step t, per batch row b):
  h1 = sigmoid(m @ A1 + pre1[t])          A1 = W_enc_w[:64]
  h2 = sigmoid(m @ B1 + pre2[t])          B1 = b_enc_w[:64]
  Wm = (h1 @ W_dec_w + W_dec_b).reshape(64,64)
  bm = h2 @ b_dec_w + b_dec_b
  m' = sigmoid(Wm @ m + bm)
  loss[t] = (logsumexp(m'@dec_w+dec_b) - (m'@dec_w+dec_b)[y]) / ln2

pre1/pre2 are the window-dependent parts, precomputed on device via a
shifted-embedding matmul.  The bilinear Wm@m is reassociated as
  a[b,i] = sum_h h1[b,h] * T[b,h,i],  T[b,h,i] = sum_j W2r[h,i,j] m[b,j]
T is produced by 32 weight-stationary matmuls (chunk c covers i=c and
i=c+32 in the two partition halves), then contracted with h1 by 4 tiny
matmuls reading strided slices of T from SBUF.

Sharding: batch rows 2k,2k+1 -> core k; zero cross-core communication.
"""

import os
import sys
import numpy as np

sys.path.insert(0, "/opt/trn_rl_repo")

import concourse.bass as bass
import concourse.bacc as bacc
import concourse.mybir as mybir
import concourse.tile as tile
from concourse.bass_utils import run_bass_kernel_spmd

import ml_dtypes

BF16 = ml_dtypes.bfloat16

Cin, E, L, M, H, Cout = 256, 16, 64, 64, 64, 256
B, N = 16, 2048
D = M + L * E  # 1088
NCORES = 8
BL = B // NCORES  # 2 batch rows per core
NB = N * BL       # 4096 (t,b) pairs per core
TAU = N + L - 8   # e8 time length: tau in [0, 2104)
E8COLS = TAU * BL  # 4208

F32 = mybir.dt.float32
BF16_DT = mybir.dt.bfloat16
FP8_DT = mybir.dt.float8e4
FP8 = ml_dtypes.float8_e4m3
AF = mybir.ActivationFunctionType

_cache = {}


def _build_nc(unroll=16, staggered=False):
    nc = bacc.Bacc("TRN2", target_bir_lowering=False, debug=True)

    # ---- DRAM parameters (per-core inputs) ----
    def P(name, shape, dt):
        return nc.declare_dram_parameter(name, list(shape), dt, isOutput=False)

    e8_d = P("e8", (128, E8COLS), BF16_DT)
    wpre1_d = P("wpre1", (128, 8 * 128), BF16_DT)
    wpre2_d = P("wpre2", (128, 8 * 64), BF16_DT)
    bias1_d = P("bias1", (1, 128), BF16_DT)   # [W_enc_b | W_enc_b]
    bias2_d = P("bias2", (1, 64), BF16_DT)    # b_enc_b
    a1b1_d = P("a1b1", (128, 192), FP8_DT)
    wstatT_d = P("wstatT", (128, 16 * 128), FP8_DT)
    wbT_d = P("wbT", (64, 64), F32)           # W_dec_b reshaped [j,i]
    bw65_d = P("bw65", (65, 64), F32)         # [b_dec_w ; b_dec_b]
    decstat_d = P("decstat", (65, 256), F32)  # [dec_w ; dec_b]
    gaug_d = P("gaug", (65, NB), F32)
    ones65_d = P("ones65", (65, 1), F32)
    ones128_d = P("ones128", (128, 1), F32)
    out_d = nc.declare_dram_parameter("out", [1, NB], F32, isOutput=True)

    with tile.TileContext(nc) as tc:
        with (
            tc.tile_pool(name="persist", bufs=1) as pp,
            tc.tile_pool(name="psum", bufs=2, space="PSUM") as psp,
        ):
            e8 = pp.tile([128, E8COLS], BF16_DT)
            wpre1 = pp.tile([128, 8 * 128], BF16_DT)
            wpre2 = pp.tile([128, 8 * 64], BF16_DT)
            bias1 = pp.tile([1, 128], BF16_DT)
            bias2 = pp.tile([1, 64], BF16_DT)
            a1b1 = pp.tile([128, 192], FP8_DT)
            wstatT = pp.tile([128, 16 * 128], FP8_DT)
            wbT = pp.tile([64, 64], F32)
            bw65 = pp.tile([65, 64], F32)
            decstat = pp.tile([65, 256], F32)
            gaug = pp.tile([65, NB], F32)
            ones65 = pp.tile([65, 1], F32)
            ones128 = pp.tile([128, 1], F32)

            for sb, dr in [
                (e8, e8_d), (wpre1, wpre1_d), (wpre2, wpre2_d),
                (bias1, bias1_d), (bias2, bias2_d), (a1b1, a1b1_d),
                (wstatT, wstatT_d), (wbT, wbT_d),
                (bw65, bw65_d), (decstat, decstat_d),
                (gaug, gaug_d), (ones65, ones65_d), (ones128, ones128_d),
            ]:
                nc.default_dma_engine.dma_start(sb[:], dr[:])

            pre1 = pp.tile([128, NB], F32)
            pre2 = pp.tile([64, NB], F32)
            m_hist = pp.tile([65, NB + 2 * BL], F32)  # row 64 == 1.0
            m_bf = pp.tile([128, BL], FP8_DT)
            h2t = pp.tile([65, BL], F32)              # row 64 == 1.0
            onerow = pp.tile([1, 512], BF16_DT)

            nc.vector.memset(m_hist[0:64, 0:BL], 0.0)
            nc.vector.memset(m_hist[64:65, :], 1.0)
            nc.vector.memset(m_bf[:], 0.0)  # both halves
            nc.vector.memset(h2t[64:65, :], 1.0)
            nc.vector.memset(onerow[:], 1.0)

            # ---- precompute pre1/pre2 ----
            for n in range(8):
                ps1 = psp.tile([128, 512], F32, tag="pps")
                for c in range(8):
                    nc.tensor.matmul(
                        ps1[:],
                        wpre1[:, c * 128:(c + 1) * 128],
                        e8[:, 16 * c + 512 * n: 16 * c + 512 * n + 512],
                        start=(c == 0), stop=False,
                    )
                nc.tensor.matmul(ps1[:], bias1[:], onerow[:],
                                 start=False, stop=True, skip_group_check=True)
                nc.vector.tensor_copy(pre1[:, 512 * n: 512 * (n + 1)], ps1[:])
                ps2 = psp.tile([64, 512], F32, tag="pps")
                for c in range(8):
                    nc.tensor.matmul(
                        ps2[:],
                        wpre2[:, c * 64:(c + 1) * 64],
                        e8[:, 16 * c + 512 * n: 16 * c + 512 * n + 512],
                        start=(c == 0), stop=False,
                    )
                nc.tensor.matmul(ps2[:], bias2[:], onerow[:],
                                 start=False, stop=True, skip_group_check=True)
                nc.vector.tensor_copy(pre2[:, 512 * n: 512 * (n + 1)], ps2[:])

            # ---- the scan ----
            with (
                tc.tile_pool(name="scan_sb", bufs=2) as wp,
                tc.For_i(0, N, unroll, staggered_reset=staggered,
                         hint_engines=(mybir.EngineType.PE,)) as iv,
            ):
                for k in range(unroll):
                    tcol = (iv + k) * BL
                    g_ps = psp.tile([128, 4], F32, tag="g_ps")
                    T_pse = psp.tile([128, 16, BL], F32, tag="T_pse", bufs=1)
                    T_pso = psp.tile([128, 16, BL], F32, tag="T_pso", bufs=1)
                    a_ps = psp.tile([64, BL], F32, tag="a_ps")
                    h1p = wp.tile([128, BL], F32, tag="h1p")
                    h2p = wp.tile([64, BL], F32, tag="h2p")
                    h1d = wp.tile([128, BL], F32, tag="h1d")
                    tsb = wp.tile([128, 32, BL], F32, tag="tsb")

                    # g row-paired: h1-preact (dup) on rows 0-63, h2 on 64-127
                    nc.tensor.matmul(g_ps[:, 0:2], a1b1[0:64, 0:128],
                                     m_bf[0:64, :], start=True, stop=True,
                                     tile_position=(0, 0))
                    nc.tensor.matmul(g_ps[0:64, 2:4], a1b1[64:128, 128:192],
                                     m_bf[64:128, :], start=True, stop=True,
                                     tile_position=(64, 0))
                    # T chunks, interleaved across row groups
                    for p2 in range(16):
                        nc.tensor.matmul(
                            T_pse[:, p2, :],
                            wstatT[0:64, p2 * 128:(p2 + 1) * 128],
                            m_bf[0:64, :], start=True, stop=True,
                            tile_position=(0, 0))
                        nc.tensor.matmul(
                            T_pso[:, p2, :],
                            wstatT[64:128, p2 * 128:(p2 + 1) * 128],
                            m_bf[64:128, :], start=True, stop=True,
                            tile_position=(64, 0))
                    # h = sigmoid(g + pre)
                    nc.vector.tensor_tensor(h1p[:], g_ps[:, 0:2],
                                            pre1[:, bass.ds(tcol, BL)],
                                            mybir.AluOpType.add)
                    nc.scalar.activation(h1d[:], h1p[:], AF.Sigmoid)
                    nc.vector.tensor_tensor(h2p[:], g_ps[0:64, 2:4],
                                            pre2[:, bass.ds(tcol, BL)],
                                            mybir.AluOpType.add)
                    nc.scalar.activation(h2t[0:64, :], h2p[:], AF.Sigmoid)
                    # T -> SBUF: tsb[:, c, :] with c = 2*p2 + odd
                    nc.vector.tensor_copy(tsb[:, 0:32:2, :], T_pse[:])
                    nc.vector.tensor_copy(tsb[:, 1:32:2, :], T_pso[:])
                    # a = WbT@m + bw65@h2 + sum_h h1*T
                    nc.tensor.matmul(a_ps[:], wbT[:],
                                     m_hist[0:64, bass.ds(tcol, BL)],
                                     start=True, stop=False)
                    nc.tensor.matmul(a_ps[:], bw65[:], h2t[:],
                                     start=False, stop=False, skip_group_check=True)
                    for b in range(BL):
                        nc.tensor.matmul(a_ps[0:32, b: b + 1],
                                         tsb[0:64, :, b], h1d[0:64, b: b + 1],
                                         start=False, stop=False,
                                         skip_group_check=True,
                                         tile_position=(0, 0))
                        last = b == BL - 1
                        nc.tensor.matmul(a_ps[32:64, b: b + 1],
                                         tsb[64:128, :, b], h1d[64:128, b: b + 1],
                                         start=False, stop=last,
                                         skip_group_check=True,
                                         tile_position=(64, 32))
                    # m' = sigmoid(a): bf16 dup halves first (critical), then f32 hist
                    nc.scalar.activation(m_bf[0:64, :], a_ps[:], AF.Sigmoid)
                    nc.scalar.activation(m_bf[64:128, :], a_ps[:], AF.Sigmoid)
                    nc.scalar.activation(m_hist[0:64, bass.ds(tcol + BL, BL)],
                                         a_ps[:], AF.Sigmoid)

            # ---- bulk loss ----
            lse = pp.tile([1, NB], F32)
            paug = pp.tile([65, NB], F32)
            loss = pp.tile([1, NB], F32)
            nc.vector.tensor_copy(paug[64:65, :], gaug[64:65, :])
            nc.vector.tensor_tensor(paug[0:64, :], gaug[0:64, :],
                                    m_hist[0:64, BL: NB + BL],
                                    mybir.AluOpType.mult)
            with tc.tile_pool(name="bulk", bufs=2) as bp:
                for tcn in range(8):
                    sl = slice(512 * tcn, 512 * (tcn + 1))
                    se_ps = psp.tile([1, 512], F32, tag="pps")
                    for half in range(2):
                        lg_ps = psp.tile([128, 512], F32, tag="pps")
                        exps = bp.tile([128, 512], F32, tag="exps")
                        nc.tensor.matmul(
                            lg_ps[:],
                            decstat[:, half * 128:(half + 1) * 128],
                            m_hist[:, BL + 512 * tcn: BL + 512 * (tcn + 1)],
                            start=True, stop=True)
                        nc.scalar.activation(exps[:], lg_ps[:], AF.Exp)
                        nc.tensor.matmul(se_ps[:], ones128[:], exps[:],
                                         start=(half == 0), stop=(half == 1))
                    nc.scalar.activation(lse[:, sl], se_ps[:], AF.Ln)
                    pk_ps = psp.tile([1, 512], F32, tag="pps")
                    nc.tensor.matmul(pk_ps[:], ones65[:], paug[:, sl],
                                     start=True, stop=True)
                    nc.vector.tensor_tensor(loss[:, sl], lse[:, sl], pk_ps[:],
                                            mybir.AluOpType.subtract)
            nc.vector.tensor_scalar_mul(loss[:], loss[:],
                                        float(1.0 / np.log(2.0)))
            nc.default_dma_engine.dma_start(out_d[:], loss[:])

    nc.compile()
    return nc


def _prep_core_inputs(x0, emb, W_enc_w, W_enc_b, W_dec_w, W_dec_b,
                      b_enc_w, b_enc_b, b_dec_w, b_dec_b, dec_w, dec_b):
    """Host-side gathers/packing -> list of per-core input dicts."""
    f32 = np.float32
    x0 = np.asarray(x0)
    xp = np.concatenate([np.zeros((B, L), x0.dtype), x0], axis=1)  # [B, N+L]
    e = np.asarray(emb, f32)[xp]  # [B, N+L, E]

    # shared weight packs
    Wcat = np.concatenate([np.asarray(W_enc_w, f32), np.asarray(b_enc_w, f32)],
                          axis=1)  # [1088, 128]
    wpre1 = np.zeros((128, 8 * 128), f32)
    wpre2 = np.zeros((128, 8 * 64), f32)
    for c in range(8):
        blk = Wcat[64 + 128 * c: 64 + 128 * (c + 1)]  # [128, 128]
        wpre1[:, c * 128: c * 128 + 64] = blk[:, :64]
        wpre1[:, c * 128 + 64: c * 128 + 128] = blk[:, :64]
        wpre2[:, c * 64:(c + 1) * 64] = blk[:, 64:]
    bias1 = np.concatenate([np.asarray(W_enc_b, f32)] * 2).reshape(1, 128)
    bias2 = np.asarray(b_enc_b, f32).reshape(1, 64)
    a1b1 = np.zeros((128, 192), f32)
    a1b1[0:64, 0:128] = np.concatenate([Wcat[:64, :64]] * 2, axis=1)
    a1b1[64:128, 128:192] = Wcat[:64, 64:]

    W2r = np.asarray(W_dec_w, f32).reshape(H, M, M)  # [h, i, j]
    wstatT = np.zeros((128, 16 * 128), f32)
    for p2 in range(16):
        for half, c in ((0, 2 * p2), (1, 2 * p2 + 1)):
            rows = slice(64 * half, 64 * half + 64)
            wstatT[rows, p2 * 128: p2 * 128 + 64] = W2r[:, c, :].T
            wstatT[rows, p2 * 128 + 64: p2 * 128 + 128] = W2r[:, c + 32, :].T
    wbT = np.asarray(W_dec_b, f32).reshape(M, M).T.copy()  # [j, i]
    bw65 = np.concatenate([np.asarray(b_dec_w, f32),
                           np.asarray(b_dec_b, f32).reshape(1, 64)], axis=0)
    decstat = np.concatenate([np.asarray(dec_w, f32),
                              np.asarray(dec_b, f32).reshape(1, 256)], axis=0)
    ones65 = np.ones((65, 1), f32)
    ones128 = np.ones((128, 1), f32)

    shared = dict(
        wpre1=wpre1.astype(BF16), wpre2=wpre2.astype(BF16),
        bias1=bias1.astype(BF16), bias2=bias2.astype(BF16),
        a1b1=a1b1.astype(FP8),
        wstatT=wstatT.astype(FP8), wbT=wbT, bw65=bw65,
        decstat=decstat, ones65=ones65, ones128=ones128,
    )

    in_maps = []
    dec_wT = np.asarray(dec_w, f32).T.copy()  # [256, 64]
    dec_bv = np.asarray(dec_b, f32)
    for k in range(NCORES):
        rows = slice(BL * k, BL * (k + 1))
        ek = e[rows]  # [BL, N+L, E]
        # e8[l_sub*16+eps, tau*BL+b] = ek[b, tau+l_sub, eps]
        e8 = np.zeros((128, E8COLS), f32)
        for ls in range(8):
            # [BL, TAU, E] -> [E? ] want [eps, tau, b]
            blk = ek[:, ls: ls + TAU, :].transpose(2, 1, 0)  # [E, TAU, BL]
            e8[ls * 16:(ls + 1) * 16] = blk.reshape(E, E8COLS)
        y = np.asarray(x0[rows])  # [BL, N]
        g = dec_wT[y]  # [BL, N, 64]
        gaug = np.zeros((65, NB), f32)
        gaug[:64] = g.transpose(2, 1, 0).reshape(64, NB)
        gaug[64] = dec_bv[y].T.reshape(NB)
        d = dict(shared)
        d["e8"] = e8.astype(BF16)
        d["gaug"] = gaug
        in_maps.append(d)
    return in_maps


def kernel(**inputs):
    key = "nc"
    if key not in _cache:
        _cache[key] = _build_nc()
    nc = _cache[key]
    in_maps = _prep_core_inputs(**inputs)
    res = run_bass_kernel_spmd(nc, in_maps, list(range(NCORES)),
                               trace=bool(os.environ.get("KERNEL_TRACE")))
    _cache["last_result"] = res
    out = np.zeros((N, B), np.float32)
    for k in range(NCORES):
        out[:, BL * k: BL * (k + 1)] = res.results[k]["out"].reshape(N, BL)
    return out.reshape(-1)



# revision 11
# speedup vs baseline: 1.0417x; 1.0411x over previous
"""Trainium2 Bass kernel for nn_Net4 (hypernetwork RNN scan).

Model (per step t, per batch row b):
  h1 = sigmoid(m @ A1 + pre1[t])          A1 = W_enc_w[:64]
  h2 = sigmoid(m @ B1 + pre2[t])          B1 = b_enc_w[:64]
  Wm = (h1 @ W_dec_w + W_dec_b).reshape(64,64)
  bm = h2 @ b_dec_w + b_dec_b
  m' = sigmoid(Wm @ m + bm)
  loss[t] = (logsumexp(m'@dec_w+dec_b) - (m'@dec_w+dec_b)[y]) / ln2

pre1/pre2 are the window-dependent parts, precomputed on device via a
shifted-embedding matmul.  The bilinear Wm@m is reassociated as
  a[b,i] = sum_h h1[b,h] * T[b,h,i],  T[b,h,i] = sum_j W2r[h,i,j] m[b,j]
T is produced by 32 weight-stationary fp8 matmuls (chunk c covers i=c
and i=c+32 in the two partition halves), then contracted with h1 by 4
small bf16 matmuls reading strided slices of T from SBUF.

All scan-phase matmuls are fp8/bf16 (fp32 matmuls run a double
LOW/HIGH pass on the PE and were the dominant tail cost).  m is kept
in two zero-padded fp8 copies (m_z1=[m;0], m_z2=[0;m]) so the packed
[128,128] stationaries (encAB, wb128) can select their row-half via
the moving operand.  b_dec_b is folded into the final sigmoid's
per-partition bias.

Sharding: batch rows 2k,2k+1 -> core k; zero cross-core communication.
"""

import os
import sys
import numpy as np

sys.path.insert(0, "/opt/trn_rl_repo")

import concourse.bass as bass
import concourse.bacc as bacc
import concourse.mybir as mybir
import concourse.tile as tile
from concourse.bass_utils import run_bass_kernel_spmd

import ml_dtypes

BF16 = ml_dtypes.bfloat16
FP8 = ml_dtypes.float8_e4m3

Cin, E, L, M, H, Cout = 256, 16, 64, 64, 64, 256
B, N = 16, 2048
D = M + L * E  # 1088
NCORES = 8
BL = B // NCORES  # 2 batch rows per core
NB = N * BL       # 4096 (t,b) pairs per core
TAU = N + L - 8   # e8 time length: tau in [0, 2104)
E8COLS = TAU * BL  # 4208

F32 = mybir.dt.float32
BF16_DT = mybir.dt.bfloat16
FP8_DT = mybir.dt.float8e4
AF = mybir.ActivationFunctionType

_cache = {}


def _build_nc(unroll=16, staggered=False):
    nc = bacc.Bacc("TRN2", target_bir_lowering=False, debug=True)

    # ---- DRAM parameters (per-core inputs) ----
    def P(name, shape, dt):
        return nc.declare_dram_parameter(name, list(shape), dt, isOutput=False)

    e8_d = P("e8", (128, E8COLS), BF16_DT)
    wpre1_d = P("wpre1", (128, 8 * 128), BF16_DT)
    wpre2_d = P("wpre2", (128, 8 * 64), BF16_DT)
    bias1_d = P("bias1", (1, 128), BF16_DT)   # [W_enc_b | W_enc_b]
    bias2_d = P("bias2", (1, 64), BF16_DT)    # b_enc_b
    encAB_d = P("encAB", (128, 128), FP8_DT)  # [[A1|A1];[B1|0]]
    wstatT_d = P("wstatT", (128, 16 * 128), FP8_DT)
    wb128_d = P("wb128", (128, 128), FP8_DT)  # [[wbT|0];[b_dec_w|0]]
    bdecb_d = P("bdecb", (64, 1), F32)        # b_dec_b (sigmoid bias)
    decstat_d = P("decstat", (65, 256), F32)  # [dec_w ; dec_b]
    gaug_d = P("gaug", (65, NB), F32)
    ones65_d = P("ones65", (65, 1), F32)
    ones128_d = P("ones128", (128, 1), F32)
    out_d = nc.declare_dram_parameter("out", [1, NB], F32, isOutput=True)

    with tile.TileContext(nc) as tc:
        with (
            tc.tile_pool(name="persist", bufs=1) as pp,
            tc.tile_pool(name="psum", bufs=2, space="PSUM") as psp,
        ):
            e8 = pp.tile([128, E8COLS], BF16_DT)
            wpre1 = pp.tile([128, 8 * 128], BF16_DT)
            wpre2 = pp.tile([128, 8 * 64], BF16_DT)
            bias1 = pp.tile([1, 128], BF16_DT)
            bias2 = pp.tile([1, 64], BF16_DT)
            encAB = pp.tile([128, 128], FP8_DT)
            wstatT = pp.tile([128, 16 * 128], FP8_DT)
            wb128 = pp.tile([128, 128], FP8_DT)
            bdecb = pp.tile([64, 1], F32)
            decstat = pp.tile([65, 256], F32)
            gaug = pp.tile([65, NB], F32)
            ones65 = pp.tile([65, 1], F32)
            ones128 = pp.tile([128, 1], F32)

            for sb, dr in [
                (e8, e8_d), (wpre1, wpre1_d), (wpre2, wpre2_d),
                (bias1, bias1_d), (bias2, bias2_d), (encAB, encAB_d),
                (wstatT, wstatT_d), (wb128, wb128_d), (bdecb, bdecb_d),
                (decstat, decstat_d),
                (gaug, gaug_d), (ones65, ones65_d), (ones128, ones128_d),
            ]:
                nc.default_dma_engine.dma_start(sb[:], dr[:])

            pre1 = pp.tile([128, NB], F32)
            pre2 = pp.tile([64, NB], F32)
            m_hist = pp.tile([65, NB + 2 * BL], F32)  # row 64 == 1.0
            m_z1 = pp.tile([128, BL], FP8_DT)   # [m ; 0]
            m_z2 = pp.tile([128, BL], FP8_DT)   # [0 ; m]
            h2z = pp.tile([128, BL], FP8_DT)    # [0 ; h2]
            h1d = pp.tile([128, BL], BF16_DT)   # [h1 ; h1]
            onerow = pp.tile([1, 512], BF16_DT)

            nc.vector.memset(m_hist[0:64, 0:BL], 0.0)
            nc.vector.memset(m_hist[64:65, :], 1.0)
            nc.vector.memset(m_z1[:], 0.0)
            nc.vector.memset(m_z2[:], 0.0)
            nc.vector.memset(h2z[:], 0.0)
            nc.vector.memset(onerow[:], 1.0)

            # ---- precompute pre1/pre2 (biases folded in) ----
            for n in range(8):
                ps1 = psp.tile([128, 512], F32, tag="pps")
                for c in range(8):
                    nc.tensor.matmul(
                        ps1[:],
                        wpre1[:, c * 128:(c + 1) * 128],
                        e8[:, 16 * c + 512 * n: 16 * c + 512 * n + 512],
                        start=(c == 0), stop=(c == 7),
                    )
                nc.tensor.matmul(ps1[:], bias1[:], onerow[:],
                                 start=False, stop=True, skip_group_check=True)
                nc.vector.tensor_copy(pre1[:, 512 * n: 512 * (n + 1)], ps1[:])
                ps2 = psp.tile([64, 512], F32, tag="pps")
                for c in range(8):
                    nc.tensor.matmul(
                        ps2[:],
                        wpre2[:, c * 64:(c + 1) * 64],
                        e8[:, 16 * c + 512 * n: 16 * c + 512 * n + 512],
                        start=(c == 0), stop=(c == 7),
                    )
                nc.tensor.matmul(ps2[:], bias2[:], onerow[:],
                                 start=False, stop=True, skip_group_check=True)
                nc.vector.tensor_copy(pre2[:, 512 * n: 512 * (n + 1)], ps2[:])

            # ---- the scan ----
            with (
                tc.tile_pool(name="scan_sb", bufs=2) as wp,
                tc.For_i(0, N, unroll, staggered_reset=staggered,
                         hint_engines=(mybir.EngineType.PE,)) as iv,
            ):
                for k in range(unroll):
                    tcol = (iv + k) * BL
                    g_ps = psp.tile([128, 2 * BL], F32, tag="g_ps")
                    T_pse = psp.tile([128, 16, BL], F32, tag="T_pse", bufs=1)
                    T_pso = psp.tile([128, 16, BL], F32, tag="T_pso", bufs=1)
                    a_ps = psp.tile([64, BL], F32, tag="a_ps")
                    h1p = wp.tile([128, BL], F32, tag="h1p")
                    h2p = wp.tile([64, BL], F32, tag="h2p")
                    tsb = wp.tile([128, 32, BL], BF16_DT, tag="tsb")

                    # --- PE stream ---
                    # g1: [A1|A1]^T m  (dup'd h1 pre-activation)
                    nc.tensor.matmul(g_ps[:, 0:BL], encAB[:], m_z1[:],
                                     start=True, stop=True)
                    # wb partial: wbT @ m  (rows 64:128 of wb128 see zeros)
                    nc.tensor.matmul(a_ps[:], wb128[:, 0:64], m_z1[:],
                                     start=True, stop=False,
                                     skip_group_check=True)
                    # T even chunks (rows 0:64 of the PE array)
                    for p2 in range(16):
                        nc.tensor.matmul(
                            T_pse[:, p2, :],
                            wstatT[0:64, p2 * 128:(p2 + 1) * 128],
                            m_z1[0:64, :], start=True, stop=True,
                            tile_position=(0, 0))
                    # g2: [B1|0]^T m  (h2 pre-activation in rows 0:64)
                    nc.tensor.matmul(g_ps[:, BL:2 * BL], encAB[:], m_z2[:],
                                     start=True, stop=True,
                                     skip_group_check=True)
                    # T odd chunks (rows 64:128)
                    for p2 in range(16):
                        nc.tensor.matmul(
                            T_pso[:, p2, :],
                            wstatT[64:128, p2 * 128:(p2 + 1) * 128],
                            m_z2[64:128, :], start=True, stop=True,
                            tile_position=(64, 0))

                    # h pre-activations and sigmoids (emitted before their
                    # PE consumers so cross-engine deps are tracked)
                    nc.vector.tensor_tensor(h1p[:], g_ps[:, 0:BL],
                                            pre1[:, bass.ds(tcol, BL)],
                                            mybir.AluOpType.add)
                    nc.vector.tensor_tensor(h2p[:], g_ps[0:64, BL:2 * BL],
                                            pre2[:, bass.ds(tcol, BL)],
                                            mybir.AluOpType.add)
                    nc.scalar.activation(h1d[:], h1p[:], AF.Sigmoid)
                    nc.scalar.activation(h2z[64:128, :], h2p[:], AF.Sigmoid)
                    # T -> SBUF (bf16): tsb[:, c, :] with c = 2*p2 + odd
                    nc.vector.tensor_copy(tsb[:, 0:32:2, :], T_pse[:])
                    nc.vector.tensor_copy(tsb[:, 1:32:2, :], T_pso[:])

                    # wb partial: b_dec_w @ h2 (rows 0:64 of wb128 see zeros)
                    nc.tensor.matmul(a_ps[:], wb128[:, 0:64], h2z[:],
                                     start=False, stop=False,
                                     skip_group_check=True)
                    # a += sum_h h1 * T  (bf16, per batch row)
                    for b in range(BL):
                        last = b == BL - 1
                        nc.tensor.matmul(a_ps[0:32, b: b + 1],
                                         tsb[0:64, :, b], h1d[0:64, b: b + 1],
                                         start=False, stop=last,
                                         skip_group_check=True,
                                         tile_position=(0, 0))
                        nc.tensor.matmul(a_ps[32:64, b: b + 1],
                                         tsb[64:128, :, b], h1d[64:128, b: b + 1],
                                         start=False, stop=last,
                                         skip_group_check=True,
                                         tile_position=(64, 32))

                    # m' = sigmoid(a + b_dec_b); fp8 dups first (critical)
                    nc.scalar.activation(m_z1[0:64, :], a_ps[0:64, :],
                                         AF.Sigmoid, bias=bdecb[:, 0:1])
                    nc.scalar.activation(m_z2[64:128, :], a_ps[0:64, :],
                                         AF.Sigmoid, bias=bdecb[:, 0:1])
                    nc.scalar.activation(m_hist[0:64, bass.ds(tcol + BL, BL)],
                                         a_ps[0:64, :],
                                         AF.Sigmoid, bias=bdecb[:, 0:1])

            # ---- bulk loss ----
            lse = pp.tile([1, NB], F32)
            paug = pp.tile([65, NB], F32)
            loss = pp.tile([1, NB], F32)
            nc.vector.tensor_copy(paug[64:65, :], gaug[64:65, :])
            nc.vector.tensor_tensor(paug[0:64, :], gaug[0:64, :],
                                    m_hist[0:64, BL: NB + BL],
                                    mybir.AluOpType.mult)
            with tc.tile_pool(name="bulk", bufs=2) as bp:
                for tcn in range(8):
                    sl = slice(512 * tcn, 512 * (tcn + 1))
                    se_ps = psp.tile([1, 512], F32, tag="pps")
                    for half in range(2):
                        lg_ps = psp.tile([128, 512], F32, tag="pps")
                        exps = bp.tile([128, 512], F32, tag="exps")
                        nc.tensor.matmul(
                            lg_ps[:],
                            decstat[:, half * 128:(half + 1) * 128],
                            m_hist[:, BL + 512 * tcn: BL + 512 * (tcn + 1)],
                            start=True, stop=True)
                        nc.scalar.activation(exps[:], lg_ps[:], AF.Exp)
                        nc.tensor.matmul(se_ps[:], ones128[:], exps[:],
                                         start=(half == 0), stop=(half == 1))
                    nc.scalar.activation(lse[:, sl], se_ps[:], AF.Ln)
                    pk_ps = psp.tile([1, 512], F32, tag="pps")
                    nc.tensor.matmul(pk_ps[:], ones65[:], paug[:, sl],
                                     start=True, stop=True)
                    nc.vector.tensor_tensor(loss[:, sl], lse[:, sl], pk_ps[:],
                                            mybir.AluOpType.subtract)
            nc.vector.tensor_scalar_mul(loss[:], loss[:],
                                        float(1.0 / np.log(2.0)))
            nc.default_dma_engine.dma_start(out_d[:], loss[:])

    nc.compile()
    return nc


def _prep_core_inputs(x0, emb, W_enc_w, W_enc_b, W_dec_w, W_dec_b,
                      b_enc_w, b_enc_b, b_dec_w, b_dec_b, dec_w, dec_b):
    """Host-side gathers/packing -> list of per-core input dicts."""
    f32 = np.float32
    x0 = np.asarray(x0)
    xp = np.concatenate([np.zeros((B, L), x0.dtype), x0], axis=1)  # [B, N+L]
    e = np.asarray(emb, f32)[xp]  # [B, N+L, E]

    # shared weight packs
    Wcat = np.concatenate([np.asarray(W_enc_w, f32), np.asarray(b_enc_w, f32)],
                          axis=1)  # [1088, 128]
    wpre1 = np.zeros((128, 8 * 128), f32)
    wpre2 = np.zeros((128, 8 * 64), f32)
    for c in range(8):
        blk = Wcat[64 + 128 * c: 64 + 128 * (c + 1)]  # [128, 128]
        wpre1[:, c * 128: c * 128 + 64] = blk[:, :64]
        wpre1[:, c * 128 + 64: c * 128 + 128] = blk[:, :64]
        wpre2[:, c * 64:(c + 1) * 64] = blk[:, 64:]
    bias1 = np.concatenate([np.asarray(W_enc_b, f32)] * 2).reshape(1, 128)
    bias2 = np.asarray(b_enc_b, f32).reshape(1, 64)
    encAB = np.zeros((128, 128), f32)
    encAB[0:64, :] = np.concatenate([Wcat[:64, :64]] * 2, axis=1)
    encAB[64:128, 0:64] = Wcat[:64, 64:]

    W2r = np.asarray(W_dec_w, f32).reshape(H, M, M)  # [h, i, j]
    wstatT = np.zeros((128, 16 * 128), f32)
    for p2 in range(16):
        for half, c in ((0, 2 * p2), (1, 2 * p2 + 1)):
            rows = slice(64 * half, 64 * half + 64)
            wstatT[rows, p2 * 128: p2 * 128 + 64] = W2r[:, c, :].T
            wstatT[rows, p2 * 128 + 64: p2 * 128 + 128] = W2r[:, c + 32, :].T
    wb128 = np.zeros((128, 128), f32)
    wb128[0:64, 0:64] = np.asarray(W_dec_b, f32).reshape(M, M).T  # [j, i]
    wb128[64:128, 0:64] = np.asarray(b_dec_w, f32)                # [h, i]
    bdecb = np.asarray(b_dec_b, f32).reshape(64, 1)
    decstat = np.concatenate([np.asarray(dec_w, f32),
                              np.asarray(dec_b, f32).reshape(1, 256)], axis=0)
    ones65 = np.ones((65, 1), f32)
    ones128 = np.ones((128, 1), f32)

    shared = dict(
        wpre1=wpre1.astype(BF16), wpre2=wpre2.astype(BF16),
        bias1=bias1.astype(BF16), bias2=bias2.astype(BF16),
        encAB=encAB.astype(FP8),
        wstatT=wstatT.astype(FP8), wb128=wb128.astype(FP8),
        bdecb=bdecb, decstat=decstat, ones65=ones65, ones128=ones128,
    )

    in_maps = []
    dec_wT = np.asarray(dec_w, f32).T.copy()  # [256, 64]
    dec_bv = np.asarray(dec_b, f32)
    for k in range(NCORES):
        rows = slice(BL * k, BL * (k + 1))
        ek = e[rows]  # [BL, N+L, E]
        # e8[l_sub*16+eps, tau*BL+b] = ek[b, tau+l_sub, eps]
        e8 = np.zeros((128, E8COLS), f32)
        for ls in range(8):
            blk = ek[:, ls: ls + TAU, :].transpose(2, 1, 0)  # [E, TAU, BL]
            e8[ls * 16:(ls + 1) * 16] = blk.reshape(E, E8COLS)
        y = np.asarray(x0[rows])  # [BL, N]
        g = dec_wT[y]  # [BL, N, 64]
        gaug = np.zeros((65, NB), f32)
        gaug[:64] = g.transpose(2, 1, 0).reshape(64, NB)
        gaug[64] = dec_bv[y].T.reshape(NB)
        d = dict(shared)
        d["e8"] = e8.astype(BF16)
        d["gaug"] = gaug
        in_maps.append(d)
    return in_maps


def kernel(**inputs):
    key = "nc"
    if key not in _cache:
        _cache[key] = _build_nc()
    nc = _cache[key]
    in_maps = _prep_core_inputs(**inputs)
    res = run_bass_kernel_spmd(nc, in_maps, list(range(NCORES)),
                               trace=bool(os.environ.get("KERNEL_TRACE")))
    _cache["last_result"] = res
    out = np.zeros((N, B), np.float32)
    for k in range(NCORES):
        out[:, BL * k: BL * (k + 1)] = res.results[k]["out"].reshape(N, BL)
    return out.reshape(-1)


# revision 12
# speedup vs baseline: 1.2960x; 1.2441x over previous
"""Trainium2 Bass kernel for nn_Net4 (hypernetwork RNN scan).

Model (per step t, per batch row b):
  h1 = sigmoid(m @ A1 + pre1[t])          A1 = W_enc_w[:64]
  h2 = sigmoid(m @ B1 + pre2[t])          B1 = b_enc_w[:64]
  Wm = (h1 @ W_dec_w + W_dec_b).reshape(64,64)
  bm = h2 @ b_dec_w + b_dec_b
  m' = sigmoid(Wm @ m + bm)
  loss[t] = (logsumexp(m'@dec_w+dec_b) - (m'@dec_w+dec_b)[y]) / ln2

pre1/pre2 are the window-dependent parts, precomputed on device via a
shifted-embedding matmul.  The bilinear Wm@m is reassociated as
  a[b,i] = sum_h h1[b,h] * T[b,h,i],  T[b,h,i] = sum_j W2r[h,i,j] m[b,j]
T is produced by 32 weight-stationary fp8 matmuls (chunk c covers i=c
and i=c+32 in the two partition halves), then contracted with h1 by 4
small bf16 matmuls reading strided slices of T from SBUF.

All scan-phase matmuls are fp8/bf16 (fp32 matmuls run a double
LOW/HIGH pass on the PE and were the dominant tail cost).  m is kept
in two zero-padded fp8 copies (m_z1=[m;0], m_z2=[0;m]) so the packed
[128,128] stationaries (encAB, wb128) can select their row-half via
the moving operand.  b_dec_b is folded into the final sigmoid's
per-partition bias.

Sharding: batch rows 2k,2k+1 -> core k; zero cross-core communication.
"""

import os
import sys
import numpy as np

sys.path.insert(0, "/opt/trn_rl_repo")

import concourse.bass as bass
import concourse.bacc as bacc
import concourse.mybir as mybir
import concourse.tile as tile
from concourse.bass_utils import run_bass_kernel_spmd

import ml_dtypes

BF16 = ml_dtypes.bfloat16
FP8 = ml_dtypes.float8_e4m3

Cin, E, L, M, H, Cout = 256, 16, 64, 64, 64, 256
B, N = 16, 2048
D = M + L * E  # 1088
NCORES = 8
BL = B // NCORES  # 2 batch rows per core
NB = N * BL       # 4096 (t,b) pairs per core
TAU = N + L - 8   # e8 time length: tau in [0, 2104)
E8COLS = TAU * BL  # 4208

F32 = mybir.dt.float32
BF16_DT = mybir.dt.bfloat16
FP8_DT = mybir.dt.float8e4
AF = mybir.ActivationFunctionType

_cache = {}


def _build_nc(unroll=16, staggered=False):
    nc = bacc.Bacc("TRN2", target_bir_lowering=False, debug=True)

    # ---- DRAM parameters (per-core inputs) ----
    def P(name, shape, dt):
        return nc.declare_dram_parameter(name, list(shape), dt, isOutput=False)

    e8_d = P("e8", (128, E8COLS), BF16_DT)
    wpre1_d = P("wpre1", (128, 8 * 128), BF16_DT)
    wpre2_d = P("wpre2", (128, 8 * 64), BF16_DT)
    bias1_d = P("bias1", (1, 128), BF16_DT)   # [W_enc_b | W_enc_b]
    bias2_d = P("bias2", (1, 64), BF16_DT)    # b_enc_b
    encAB_d = P("encAB", (128, 128), FP8_DT)  # [[A1|A1];[B1|0]]
    wstatT_d = P("wstatT", (128, 16 * 128), FP8_DT)
    wb128_d = P("wb128", (128, 128), FP8_DT)  # [[wbT|0];[b_dec_w|0]]
    bdecb_d = P("bdecb", (64, 1), F32)        # b_dec_b (sigmoid bias)
    decstat_d = P("decstat", (65, 256), F32)  # [dec_w ; dec_b]
    gaug_d = P("gaug", (65, NB), F32)
    ones65_d = P("ones65", (65, 1), F32)
    ones128_d = P("ones128", (128, 1), F32)
    out_d = nc.declare_dram_parameter("out", [1, NB], F32, isOutput=True)

    with tile.TileContext(nc) as tc:
        with (
            tc.tile_pool(name="persist", bufs=1) as pp,
            tc.tile_pool(name="psum", bufs=2, space="PSUM") as psp,
        ):
            e8 = pp.tile([128, E8COLS], BF16_DT)
            wpre1 = pp.tile([128, 8 * 128], BF16_DT)
            wpre2 = pp.tile([128, 8 * 64], BF16_DT)
            bias1 = pp.tile([1, 128], BF16_DT)
            bias2 = pp.tile([1, 64], BF16_DT)
            encAB = pp.tile([128, 128], FP8_DT)
            wstatT = pp.tile([128, 16 * 128], FP8_DT)
            wb128 = pp.tile([128, 128], FP8_DT)
            bdecb = pp.tile([64, 1], F32)
            decstat = pp.tile([65, 256], F32)
            gaug = pp.tile([65, NB], F32)
            ones65 = pp.tile([65, 1], F32)
            ones128 = pp.tile([128, 1], F32)

            for sb, dr in [
                (e8, e8_d), (wpre1, wpre1_d), (wpre2, wpre2_d),
                (bias1, bias1_d), (bias2, bias2_d), (encAB, encAB_d),
                (wstatT, wstatT_d), (wb128, wb128_d), (bdecb, bdecb_d),
                (decstat, decstat_d),
                (gaug, gaug_d), (ones65, ones65_d), (ones128, ones128_d),
            ]:
                nc.default_dma_engine.dma_start(sb[:], dr[:])

            pre1 = pp.tile([128, NB], F32)
            pre2 = pp.tile([64, NB], F32)
            m_hist = pp.tile([65, NB + 2 * BL], F32)  # row 64 == 1.0
            m_z1 = pp.tile([128, BL], FP8_DT)   # [m ; 0]
            m_z2 = pp.tile([128, BL], FP8_DT)   # [0 ; m]
            h2z = pp.tile([128, BL], FP8_DT)    # [0 ; h2]
            h1d = pp.tile([128, BL], BF16_DT)   # [h1 ; h1]
            onerow = pp.tile([1, 512], BF16_DT)

            nc.vector.memset(m_hist[0:64, 0:BL], 0.0)
            nc.vector.memset(m_hist[64:65, :], 1.0)
            nc.vector.memset(m_z1[:], 0.0)
            nc.vector.memset(m_z2[:], 0.0)
            nc.vector.memset(h2z[:], 0.0)
            nc.vector.memset(onerow[:], 1.0)

            # ---- precompute pre1/pre2 (biases folded in) ----
            for n in range(8):
                ps1 = psp.tile([128, 512], F32, tag="pps")
                for c in range(8):
                    nc.tensor.matmul(
                        ps1[:],
                        wpre1[:, c * 128:(c + 1) * 128],
                        e8[:, 16 * c + 512 * n: 16 * c + 512 * n + 512],
                        start=(c == 0), stop=(c == 7),
                    )
                nc.tensor.matmul(ps1[:], bias1[:], onerow[:],
                                 start=False, stop=True, skip_group_check=True)
                nc.vector.tensor_copy(pre1[:, 512 * n: 512 * (n + 1)], ps1[:])
                ps2 = psp.tile([64, 512], F32, tag="pps")
                for c in range(8):
                    nc.tensor.matmul(
                        ps2[:],
                        wpre2[:, c * 64:(c + 1) * 64],
                        e8[:, 16 * c + 512 * n: 16 * c + 512 * n + 512],
                        start=(c == 0), stop=(c == 7),
                    )
                nc.tensor.matmul(ps2[:], bias2[:], onerow[:],
                                 start=False, stop=True, skip_group_check=True)
                nc.vector.tensor_copy(pre2[:, 512 * n: 512 * (n + 1)], ps2[:])

            # ---- the scan ----
            with (
                tc.tile_pool(name="scan_sb", bufs=2) as wp,
                tc.For_i(0, N, unroll, staggered_reset=staggered,
                         hint_engines=(mybir.EngineType.PE,)) as iv,
            ):
                for k in range(unroll):
                    tcol = (iv + k) * BL
                    g1_ps = psp.tile([128, BL], F32, tag="g1_ps", bufs=1)
                    g2_ps = psp.tile([128, BL], F32, tag="g2_ps", bufs=1)
                    T_pse = psp.tile([128, 16, BL], F32, tag="T_pse", bufs=1)
                    T_pso = psp.tile([128, 16, BL], F32, tag="T_pso", bufs=1)
                    a_ps = psp.tile([64, BL], F32, tag="a_ps")
                    h1p = wp.tile([128, BL], F32, tag="h1p")
                    h2p = wp.tile([64, BL], F32, tag="h2p")
                    tsb = wp.tile([128, 32, BL], BF16_DT, tag="tsb")

                    # --- PE stream ---
                    # g1: [A1|A1]^T m  (dup'd h1 pre-activation)
                    nc.tensor.matmul(g1_ps[:], encAB[:], m_z1[:],
                                     start=True, stop=True)
                    # wb partial: wbT @ m  (rows 64:128 of wb128 see zeros)
                    nc.tensor.matmul(a_ps[:], wb128[:, 0:64], m_z1[:],
                                     start=True, stop=False,
                                     skip_group_check=True)
                    # T even chunks (rows 0:64 of the PE array)
                    for p2 in range(16):
                        nc.tensor.matmul(
                            T_pse[:, p2, :],
                            wstatT[0:64, p2 * 128:(p2 + 1) * 128],
                            m_z1[0:64, :], start=True, stop=True,
                            tile_position=(0, 0))
                    # g2: [B1|0]^T m  (h2 pre-activation in rows 0:64)
                    nc.tensor.matmul(g2_ps[:], encAB[:], m_z2[:],
                                     start=True, stop=True)
                    # T odd chunks (rows 64:128)
                    for p2 in range(16):
                        nc.tensor.matmul(
                            T_pso[:, p2, :],
                            wstatT[64:128, p2 * 128:(p2 + 1) * 128],
                            m_z2[64:128, :], start=True, stop=True,
                            tile_position=(64, 0))

                    # h pre-activations and sigmoids (emitted before their
                    # PE consumers so cross-engine deps are tracked)
                    nc.vector.tensor_tensor(h1p[:], g1_ps[:],
                                            pre1[:, bass.ds(tcol, BL)],
                                            mybir.AluOpType.add)
                    nc.vector.tensor_tensor(h2p[:], g2_ps[0:64, :],
                                            pre2[:, bass.ds(tcol, BL)],
                                            mybir.AluOpType.add)
                    nc.scalar.activation(h1d[:], h1p[:], AF.Sigmoid)
                    nc.scalar.activation(h2z[64:128, :], h2p[:], AF.Sigmoid)
                    # T -> SBUF (bf16): tsb[:, c, :] with c = 2*p2 + odd
                    nc.vector.tensor_copy(tsb[:, 0:32:2, :], T_pse[:])
                    nc.vector.tensor_copy(tsb[:, 1:32:2, :], T_pso[:])

                    # wb partial: b_dec_w @ h2 (rows 0:64 of wb128 see zeros)
                    nc.tensor.matmul(a_ps[:], wb128[:, 0:64], h2z[:],
                                     start=False, stop=False,
                                     skip_group_check=True)
                    # a += sum_h h1 * T  (bf16, per batch row)
                    for b in range(BL):
                        last = b == BL - 1
                        nc.tensor.matmul(a_ps[0:32, b: b + 1],
                                         tsb[0:64, :, b], h1d[0:64, b: b + 1],
                                         start=False, stop=last,
                                         skip_group_check=True,
                                         tile_position=(0, 0))
                        nc.tensor.matmul(a_ps[32:64, b: b + 1],
                                         tsb[64:128, :, b], h1d[64:128, b: b + 1],
                                         start=False, stop=last,
                                         skip_group_check=True,
                                         tile_position=(64, 32))

                    # m' = sigmoid(a + b_dec_b); fp8 dups first (critical)
                    nc.scalar.activation(m_z1[0:64, :], a_ps[0:64, :],
                                         AF.Sigmoid, bias=bdecb[:, 0:1])
                    nc.vector.tensor_copy(m_z2[64:128, :], m_z1[0:64, :])
                    nc.scalar.activation(m_hist[0:64, bass.ds(tcol + BL, BL)],
                                         a_ps[0:64, :],
                                         AF.Sigmoid, bias=bdecb[:, 0:1])

            # ---- bulk loss ----
            lse = pp.tile([1, NB], F32)
            paug = pp.tile([65, NB], F32)
            loss = pp.tile([1, NB], F32)
            nc.vector.tensor_copy(paug[64:65, :], gaug[64:65, :])
            nc.vector.tensor_tensor(paug[0:64, :], gaug[0:64, :],
                                    m_hist[0:64, BL: NB + BL],
                                    mybir.AluOpType.mult)
            with tc.tile_pool(name="bulk", bufs=2) as bp:
                for tcn in range(8):
                    sl = slice(512 * tcn, 512 * (tcn + 1))
                    se_ps = psp.tile([1, 512], F32, tag="pps")
                    for half in range(2):
                        lg_ps = psp.tile([128, 512], F32, tag="pps")
                        exps = bp.tile([128, 512], F32, tag="exps")
                        nc.tensor.matmul(
                            lg_ps[:],
                            decstat[:, half * 128:(half + 1) * 128],
                            m_hist[:, BL + 512 * tcn: BL + 512 * (tcn + 1)],
                            start=True, stop=True)
                        nc.scalar.activation(exps[:], lg_ps[:], AF.Exp)
                        nc.tensor.matmul(se_ps[:], ones128[:], exps[:],
                                         start=(half == 0), stop=(half == 1))
                    nc.scalar.activation(lse[:, sl], se_ps[:], AF.Ln)
                    pk_ps = psp.tile([1, 512], F32, tag="pps")
                    nc.tensor.matmul(pk_ps[:], ones65[:], paug[:, sl],
                                     start=True, stop=True)
                    nc.vector.tensor_tensor(loss[:, sl], lse[:, sl], pk_ps[:],
                                            mybir.AluOpType.subtract)
            nc.vector.tensor_scalar_mul(loss[:], loss[:],
                                        float(1.0 / np.log(2.0)))
            nc.default_dma_engine.dma_start(out_d[:], loss[:])

    nc.compile()
    return nc


def _prep_core_inputs(x0, emb, W_enc_w, W_enc_b, W_dec_w, W_dec_b,
                      b_enc_w, b_enc_b, b_dec_w, b_dec_b, dec_w, dec_b):
    """Host-side gathers/packing -> list of per-core input dicts."""
    f32 = np.float32
    x0 = np.asarray(x0)
    xp = np.concatenate([np.zeros((B, L), x0.dtype), x0], axis=1)  # [B, N+L]
    e = np.asarray(emb, f32)[xp]  # [B, N+L, E]

    # shared weight packs
    Wcat = np.concatenate([np.asarray(W_enc_w, f32), np.asarray(b_enc_w, f32)],
                          axis=1)  # [1088, 128]
    wpre1 = np.zeros((128, 8 * 128), f32)
    wpre2 = np.zeros((128, 8 * 64), f32)
    for c in range(8):
        blk = Wcat[64 + 128 * c: 64 + 128 * (c + 1)]  # [128, 128]
        wpre1[:, c * 128: c * 128 + 64] = blk[:, :64]
        wpre1[:, c * 128 + 64: c * 128 + 128] = blk[:, :64]
        wpre2[:, c * 64:(c + 1) * 64] = blk[:, 64:]
    bias1 = np.concatenate([np.asarray(W_enc_b, f32)] * 2).reshape(1, 128)
    bias2 = np.asarray(b_enc_b, f32).reshape(1, 64)
    encAB = np.zeros((128, 128), f32)
    encAB[0:64, :] = np.concatenate([Wcat[:64, :64]] * 2, axis=1)
    encAB[64:128, 0:64] = Wcat[:64, 64:]

    W2r = np.asarray(W_dec_w, f32).reshape(H, M, M)  # [h, i, j]
    wstatT = np.zeros((128, 16 * 128), f32)
    for p2 in range(16):
        for half, c in ((0, 2 * p2), (1, 2 * p2 + 1)):
            rows = slice(64 * half, 64 * half + 64)
            wstatT[rows, p2 * 128: p2 * 128 + 64] = W2r[:, c, :].T
            wstatT[rows, p2 * 128 + 64: p2 * 128 + 128] = W2r[:, c + 32, :].T
    wb128 = np.zeros((128, 128), f32)
    wb128[0:64, 0:64] = np.asarray(W_dec_b, f32).reshape(M, M).T  # [j, i]
    wb128[64:128, 0:64] = np.asarray(b_dec_w, f32)                # [h, i]
    bdecb = np.asarray(b_dec_b, f32).reshape(64, 1)
    decstat = np.concatenate([np.asarray(dec_w, f32),
                              np.asarray(dec_b, f32).reshape(1, 256)], axis=0)
    ones65 = np.ones((65, 1), f32)
    ones128 = np.ones((128, 1), f32)

    shared = dict(
        wpre1=wpre1.astype(BF16), wpre2=wpre2.astype(BF16),
        bias1=bias1.astype(BF16), bias2=bias2.astype(BF16),
        encAB=encAB.astype(FP8),
        wstatT=wstatT.astype(FP8), wb128=wb128.astype(FP8),
        bdecb=bdecb, decstat=decstat, ones65=ones65, ones128=ones128,
    )

    in_maps = []
    dec_wT = np.asarray(dec_w, f32).T.copy()  # [256, 64]
    dec_bv = np.asarray(dec_b, f32)
    for k in range(NCORES):
        rows = slice(BL * k, BL * (k + 1))
        ek = e[rows]  # [BL, N+L, E]
        # e8[l_sub*16+eps, tau*BL+b] = ek[b, tau+l_sub, eps]
        e8 = np.zeros((128, E8COLS), f32)
        for ls in range(8):
            blk = ek[:, ls: ls + TAU, :].transpose(2, 1, 0)  # [E, TAU, BL]
            e8[ls * 16:(ls + 1) * 16] = blk.reshape(E, E8COLS)
        y = np.asarray(x0[rows])  # [BL, N]
        g = dec_wT[y]  # [BL, N, 64]
        gaug = np.zeros((65, NB), f32)
        gaug[:64] = g.transpose(2, 1, 0).reshape(64, NB)
        gaug[64] = dec_bv[y].T.reshape(NB)
        d = dict(shared)
        d["e8"] = e8.astype(BF16)
        d["gaug"] = gaug
        in_maps.append(d)
    return in_maps


def kernel(**inputs):
    key = "nc"
    if key not in _cache:
        _cache[key] = _build_nc()
    nc = _cache[key]
    in_maps = _prep_core_inputs(**inputs)
    res = run_bass_kernel_spmd(nc, in_maps, list(range(NCORES)),
                               trace=bool(os.environ.get("KERNEL_TRACE")))
    _cache["last_result"] = res
    out = np.zeros((N, B), np.float32)
    for k in range(NCORES):
        out[:, BL * k: BL * (k + 1)] = res.results[k]["out"].reshape(N, BL)
    return out.reshape(-1)
